# revision 1
# baseline (speedup 1.0000x reference)
"""Trainium2 Bass kernel for nn_Meta_67078799229377 (relation-network meta-learner).

Sharding: 8 cores = 4 batch elements x 2 halves of the relation-j axis.
Each core runs the full backbone for its batch element's 6 images, then the
relation network for its 18 (i, j) pairs, fully fused on-chip (the
[s,s,m,m,128] tensor never exists in HBM). Host code only reshapes/shards
inputs and combines 144 scores + 24 per-sample CE terms into the 3 scalar
losses.
"""
import os
import numpy as np
import ml_dtypes

import concourse.bass as bass
import concourse.mybir as mybir
import concourse.tile as tile
from concourse import bacc
from concourse.bass_utils import run_bass_kernel_spmd

F32 = mybir.dt.float32
F32R = mybir.dt.float32r
BF16 = mybir.dt.bfloat16
AF = mybir.ActivationFunctionType
OP = mybir.AluOpType

B, S, D = 4, 6, 8
M = D * D            # 64 spatial positions
C2 = 66              # 64 channels + 2 coord channels
H1 = 128             # g-MLP hidden
CO = 64              # g-MLP out
NCls = 64
N_CORES = 8

# Fraction of hdd-gen ops routed to the scalar engine (rest on vector engine).
ACT_HDD_EVERY = 5    # every 5th q goes to ACT


def _build_nc():
    nc = bacc.Bacc("TRN2", target_bir_lowering=False, debug=False,
                   num_devices=N_CORES)

    din = {}
    def dram_in(name, shape, dtype=F32):
        din[name] = nc.dram_tensor(name, list(shape), dtype, kind="ExternalInput")
        return din[name]

    x_patches = dram_in("patches", [27, S, 1024], BF16)
    x_w1 = dram_in("w1", [27, 32], BF16)
    x_w2 = dram_in("w2", [32, 9 * 48], BF16)
    x_w3 = dram_in("w3", [48, 9 * 64], BF16)
    x_bc1 = dram_in("bc1", [32, 1])
    x_bc2 = dram_in("bc2", [48, 1])
    x_bc3 = dram_in("bc3", [64, 1])
    x_coords = dram_in("coords", [2, S * M], BF16)
    x_wle = dram_in("wle", [65, NCls])
    x_onehot = dram_in("onehot", [S, NCls])
    x_w1a = dram_in("w1a", [C2, H1], BF16)
    x_w1b = dram_in("w1b", [C2, H1], BF16)
    x_bg1 = dram_in("bg1", [H1, 1])
    x_wg2 = dram_in("wg2", [H1, CO], BF16)
    x_bg2 = dram_in("bg2_2", [2 * CO, 1])
    x_wf1e = dram_in("wf1e", [65, 16])
    x_wf2e = dram_in("wf2e", [17, 1])

    out_scores = nc.dram_tensor("scores", [18, 1], F32, kind="ExternalOutput")
    out_cls = nc.dram_tensor("clsv", [S, 1], F32, kind="ExternalOutput")

    with tile.TileContext(nc) as tc:
        with (
            tc.tile_pool(name="const", bufs=1) as cpool,
            tc.tile_pool(name="work", bufs=1) as wpool,
            tc.tile_pool(name="patch", bufs=1) as ppool,
            tc.tile_pool(name="hdd", bufs=2) as hpool,
            tc.tile_pool(name="gscr", bufs=2) as spool,
            tc.tile_pool(name="pconv", bufs=2, space="PSUM") as pc_pool,
            tc.tile_pool(name="pbig", bufs=2, space="PSUM") as pb_pool,
            tc.tile_pool(name="psmall", bufs=2, space="PSUM") as ps_pool,
        ):
            # ---- constants to SBUF ----
            def c_tile(src, shape, dtype=F32):
                t = cpool.tile(list(shape), dtype, tag=src.name)
                nc.sync.dma_start(out=t[:], in_=src[:])
                return t

            w1_sb = c_tile(x_w1, [27, 32], BF16)
            w2_sb = c_tile(x_w2, [32, 9 * 48], BF16)
            w3_sb = c_tile(x_w3, [48, 9 * 64], BF16)
            bc1_sb = c_tile(x_bc1, [32, 1])
            bc2_sb = c_tile(x_bc2, [48, 1])
            bc3_sb = c_tile(x_bc3, [64, 1])
            wle_sb = c_tile(x_wle, [65, NCls])
            onehot_sb = c_tile(x_onehot, [S, NCls])
            w1a_sb = c_tile(x_w1a, [C2, H1], BF16)
            w1b_sb = c_tile(x_w1b, [C2, H1], BF16)
            bg1_sb = c_tile(x_bg1, [H1, 1])
            wg2_sb = c_tile(x_wg2, [H1, CO], BF16)
            bg2_sb = c_tile(x_bg2, [2 * CO, 1])
            wf1e_sb = c_tile(x_wf1e, [65, 16])
            wf2e_sb = c_tile(x_wf2e, [17, 1])

            patches_sb = ppool.tile([27, S, 1024], BF16)
            nc.sync.dma_start(out=patches_sb[:], in_=x_patches[:])

            featc = wpool.tile([C2, S * M], BF16)
            nc.sync.dma_start(out=featc[64:66, :], in_=x_coords[:])

            def r32(ap):
                return ap

            _stages = ["c1", "c2", "c3", "cls", "uv", "rel", "full"]
            _stop = os.environ.get("KSTOP", "full")
            def _do(stage):
                return _stages.index(stage) <= _stages.index(_stop)


            # ---- conv1: [27]->[32], 64x64 -> 32x32 (stride 2, im2col'd) ----
            c1sb = wpool.tile([32, S, 33, 33], BF16)
            for img in range(S):
                # zero the padding strip (row 32 and col 32)
                nc.gpsimd.memset(c1sb[:, img, 32, :], 0.0)
                nc.gpsimd.memset(c1sb[:, img, 0:32, 32], 0.0)
            for img in range(S):
                for h in range(2):
                    ps1 = pc_pool.tile([32, 16, 32], F32, tag="psc")
                    nc.tensor.matmul(
                        ps1[:].rearrange("p a b -> p (a b)"),
                        r32(w1_sb[:]),
                        r32(patches_sb[:, img, h * 512:(h + 1) * 512]),
                        start=True, stop=True)
                    # relu(x + bc1) -> padded layout; alternate engines
                    out_ap = c1sb[:, img, h * 16:(h + 1) * 16, 0:32]
                    if img % 2 == 0:
                        nc.scalar.activation(out_ap, ps1[:], AF.Relu, bias=bc1_sb[:])
                    else:
                        nc.vector.tensor_scalar(out_ap, ps1[:], bc1_sb[:], 0.0,
                                                op0=OP.add, op1=OP.max)

            if _do("c2"):
                # ---- conv2: [32]->[48], 32x32 -> 16x16 ----
                c2sb = wpool.tile([48, S, 17, 17], BF16)
                for img in range(S):
                    nc.gpsimd.memset(c2sb[:, img, 16, :], 0.0)
                    nc.gpsimd.memset(c2sb[:, img, 0:16, 16], 0.0)
                for ip in range(3):      # image pairs
                    ps2 = pc_pool.tile([48, 2, 16, 16], F32, tag="psc")
                    for k, (dy, dx) in enumerate((dy, dx) for dy in range(3) for dx in range(3)):
                        nc.tensor.matmul(
                            ps2[:],
                            r32(w2_sb[:, k * 48:(k + 1) * 48]),
                            r32(c1sb[:, 2 * ip:2 * ip + 2, dy:dy + 31:2, dx:dx + 31:2]),
                            start=(k == 0), stop=(k == 8))
                    out_ap = c2sb[:, 2 * ip:2 * ip + 2, 0:16, 0:16]
                    if ip % 2 == 0:
                        nc.scalar.activation(out_ap, ps2[:], AF.Relu, bias=bc2_sb[:])
                    else:
                        nc.vector.tensor_scalar(out_ap, ps2[:], bc2_sb[:], 0.0,
                                                op0=OP.add, op1=OP.max)

            if _do("c3"):
                # ---- conv3: [48]->[64], 16x16 -> 8x8 ----
                ps3 = ps_pool.tile([64, S, D, D], F32, tag="sm")
                for k, (dy, dx) in enumerate((dy, dx) for dy in range(3) for dx in range(3)):
                    nc.tensor.matmul(
                        ps3[:],
                        r32(w3_sb[:, k * 64:(k + 1) * 64]),
                        r32(c2sb[:, :, dy:dy + 15:2, dx:dx + 15:2]),
                        start=(k == 0), stop=(k == 8))
                nc.scalar.activation(featc[0:64, :].rearrange("p (i m) -> p i m", m=M),
                                     ps3[:].rearrange("p i a b -> p i (a b)"),
                                     AF.Relu, bias=bc3_sb[:])

            if _do("cls"):
                # ---- cls head ----
                fme = wpool.tile([65, S], F32)
                nc.gpsimd.memset(fme[:], 1.0)
                nc.vector.tensor_reduce(
                    fme[0:64, :], featc[0:64, :].rearrange("p (i m) -> p i m", m=M),
                    axis=mybir.AxisListType.X, op=OP.add)
                psl = ps_pool.tile([S, NCls], F32, tag="sm")
                nc.tensor.matmul(psl[:], r32(fme[:]), r32(wle_sb[:]), start=True, stop=True)
                mx = wpool.tile([S, 1], F32)
                nc.vector.tensor_reduce(mx[:], psl[:], axis=mybir.AxisListType.X, op=OP.max)
                shifted = wpool.tile([S, NCls], F32)
                nc.vector.tensor_scalar(shifted[:], psl[:], mx[:], None, op0=OP.subtract)
                escr = wpool.tile([S, NCls], F32)
                se = wpool.tile([S, 1], F32)
                nc.scalar.activation(escr[:], shifted[:], AF.Exp, accum_out=se[:])
                lse = wpool.tile([S, 1], F32)
                nc.scalar.activation(lse[:], se[:], AF.Ln)
                selscr = wpool.tile([S, NCls], F32)
                sel = wpool.tile([S, 1], F32)
                nc.vector.tensor_tensor(selscr[:], shifted[:], onehot_sb[:], op=OP.mult)
                nc.vector.tensor_reduce(sel[:], selscr[:], axis=mybir.AxisListType.X, op=OP.add)
                clsv = wpool.tile([S, 1], F32)
                nc.vector.tensor_tensor(clsv[:], lse[:], sel[:], op=OP.subtract)
                nc.sync.dma_start(out=out_cls[:], in_=clsv[:])

            if _do("uv"):
                # ---- u / v ----
                psu = ps_pool.tile([H1, S * M], F32, tag="sm")
                psv = ps_pool.tile([H1, S * M], F32, tag="sm")
                nc.tensor.matmul(psu[:], r32(w1a_sb[:]), r32(featc[:]), start=True, stop=True)
                nc.tensor.matmul(psv[:], r32(w1b_sb[:]), r32(featc[:]), start=True, stop=True)
                u_f32 = wpool.tile([H1, S * M], F32)
                v_bf = wpool.tile([H1, S * M], BF16)
                v_f32 = wpool.tile([H1, S * M], F32)
                nc.scalar.activation(u_f32[:], psu[:], AF.Copy)
                nc.vector.tensor_scalar(v_bf[:], psv[:], bg1_sb[:], None, op0=OP.add)
                nc.vector.tensor_scalar(v_f32[:], psv[:], bg1_sb[:], None, op0=OP.add)

            if _do("rel"):
                # ---- relation stage ----
                xf_cols = wpool.tile([2 * CO, 36], F32)
                nc.gpsimd.memset(xf_cols[:], 0.0)
                max_units = int(os.environ.get("KUNITS", "6"))
                unit_no = 0
                for jl in range(3):
                    for qh in range(2):
                        unit_no += 1
                        if unit_no > max_units:
                            continue
                        hdd = hpool.tile([H1, 32, S * M], BF16, tag="hdd")
                        for ql in range(32):
                            q = qh * 32 + ql
                            ucol = u_f32[:, jl * M + q: jl * M + q + 1]
                            if ql % ACT_HDD_EVERY == ACT_HDD_EVERY - 1:
                                nc.scalar.activation(hdd[:, ql, :], v_f32[:],
                                                     AF.Relu, bias=ucol)
                            else:
                                nc.vector.tensor_scalar(hdd[:, ql, :], v_bf[:],
                                                        ucol, 0.0,
                                                        op0=OP.add, op1=OP.max)
                        for duo in range(3):
                            iA, iB = 2 * duo, 2 * duo + 1
                            for gh in range(2):
                                ps = pb_pool.tile([2 * CO, 1024], F32, tag="gps")
                                for q2 in range(2):
                                    qg = gh * 2 + q2
                                    nc.tensor.matmul(
                                        ps[0:CO, q2 * 512:(q2 + 1) * 512],
                                        wg2_sb[:],
                                        hdd[:, qg * 8:(qg + 1) * 8, iA * M:(iA + 1) * M],
                                        start=True, stop=True)
                                    nc.tensor.matmul(
                                        ps[CO:2 * CO, q2 * 512:(q2 + 1) * 512],
                                        wg2_sb[:],
                                        hdd[:, qg * 8:(qg + 1) * 8, iB * M:(iB + 1) * M],
                                        start=True, stop=True,
                                        tile_position=(0, 64))
                                ucol_i = (((jl * 2 + qh) * 3 + duo) * 2) + gh
                                gscr = spool.tile([2 * CO, 1024], BF16, tag="gscr")
                                nc.scalar.activation(gscr[:], ps[:], AF.Relu,
                                                     bias=bg2_sb[:],
                                                     accum_out=xf_cols[:, ucol_i:ucol_i + 1])

            if _do("rel"):
                # ---- score head ----
                # sum the two gh-halves, then the two qh-halves
                xf18 = wpool.tile([2 * CO, 18], F32)
                nc.vector.tensor_tensor(
                    xf18[:],
                    xf_cols[:].rearrange("p (a g) -> p a g", g=2)[:, :, 0],
                    xf_cols[:].rearrange("p (a g) -> p a g", g=2)[:, :, 1],
                    op=OP.add)
                # xf_pair[:, jl*3+d] = xf18[:, jl*6+d] + xf18[:, jl*6+3+d]
                xf_pair = wpool.tile([2 * CO, 3, 3], F32)
                nc.vector.tensor_tensor(
                    xf_pair[:],
                    xf18[:].rearrange("p (a b) -> p a b", a=6)[:, 0:6:2, :],
                    xf18[:].rearrange("p (a b) -> p a b", a=6)[:, 1:6:2, :],
                    op=OP.add)
                xf_ext = wpool.tile([65, 18], F32)
                nc.gpsimd.memset(xf_ext[:], 1.0)
                # even local-pair columns <- partitions 0:64 (i = 2d)
                nc.vector.tensor_copy(
                    xf_ext[0:64, :].rearrange("p (a b) -> p a b", a=3)[:, :, 0:6:2],
                    xf_pair[0:64, :, :])
                # odd local-pair columns <- partitions 64:128 (i = 2d+1), needs DMA
                nc.sync.dma_start(
                    out=xf_ext[0:64, :].rearrange("p (a b) -> p a b", a=3)[:, :, 1:6:2],
                    in_=xf_pair[64:128, :, :])
                psh1 = ps_pool.tile([16, 18], F32, tag="sm")
                nc.tensor.matmul(psh1[:], r32(wf1e_sb[:]), r32(xf_ext[:]),
                                 start=True, stop=True)
                h1e = wpool.tile([17, 18], F32)
                nc.gpsimd.memset(h1e[:], 1.0)
                nc.scalar.activation(h1e[0:16, :], psh1[:], AF.Relu)
                psh2 = ps_pool.tile([18, 1], F32, tag="sm")
                nc.tensor.matmul(psh2[:], r32(h1e[:]), r32(wf2e_sb[:]),
                                 start=True, stop=True)
                en = wpool.tile([18, 1], F32)
                nc.scalar.activation(en[:], psh2[:], AF.Exp, scale=-1.0)
                ep1 = wpool.tile([18, 1], F32)
                nc.vector.tensor_scalar(ep1[:], en[:], 1.0, None, op0=OP.add)
                sc = wpool.tile([18, 1], F32)
                nc.vector.reciprocal(sc[:], ep1[:])
                nc.sync.dma_start(out=out_scores[:], in_=sc[:])

            if not _do("cls"):
                d2 = wpool.tile([S, 1], F32, tag="dummy2")
                nc.gpsimd.memset(d2[:], 0.0)
                nc.sync.dma_start(out=out_cls[:], in_=d2[:])
            if not _do("rel"):
                d1 = wpool.tile([18, 1], F32, tag="dummy1")
                nc.gpsimd.memset(d1[:], 0.0)
                nc.sync.dma_start(out=out_scores[:], in_=d1[:])
    nc.compile()
    return nc


_NC_CACHE = None


def _get_nc():
    global _NC_CACHE
    if _NC_CACHE is None:
        _NC_CACHE = _build_nc()
    return _NC_CACHE


def _host_prep(inputs):
    ins = {k: np.asarray(v) for k, v in inputs.items()}
    x = np.concatenate([ins['support_x'], ins['query_x']], axis=1)
    lab = np.concatenate([ins['support_y'], ins['query_y']], axis=1)

    xpad = np.pad(x.astype(np.float32), ((0, 0), (0, 0), (0, 0), (0, 1), (0, 1)))
    win = np.lib.stride_tricks.sliding_window_view(xpad, (3, 3), axis=(3, 4))
    win = win[:, :, :, ::2, ::2]
    patches = win.transpose(0, 2, 5, 6, 1, 3, 4).reshape(B, 27, S, 1024)
    patches = np.ascontiguousarray(patches, np.float32)

    f32 = np.float32
    bf16 = ml_dtypes.bfloat16
    w1 = np.ascontiguousarray(ins['k1'].reshape(32, 27).T, f32).astype(bf16)
    w2 = np.ascontiguousarray(ins['k2'].transpose(1, 2, 3, 0).reshape(32, 9 * 48), f32).astype(bf16)
    w3 = np.ascontiguousarray(ins['k3'].transpose(1, 2, 3, 0).reshape(48, 9 * 64), f32).astype(bf16)

    ii = np.arange(D, dtype=f32) / D
    coord = np.stack([np.broadcast_to(ii[:, None], (D, D)),
                      np.broadcast_to(ii[None, :], (D, D))]).reshape(2, M)
    coords = np.ascontiguousarray(np.tile(coord, (1, S)), f32).astype(bf16)

    onehots = np.zeros((B, S, NCls), f32)
    for b in range(B):
        onehots[b, np.arange(S), lab[b]] = 1.0

    Wg1 = ins['Wg1'].astype(f32)
    common = dict(
        w1=w1, w2=w2, w3=w3,
        bc1=np.ascontiguousarray(ins['bc1'].reshape(32, 1), f32),
        bc2=np.ascontiguousarray(ins['bc2'].reshape(48, 1), f32),
        bc3=np.ascontiguousarray(ins['bc3'].reshape(64, 1), f32),
        coords=coords,
        wle=np.ascontiguousarray(
            np.vstack([ins['Wlog'].astype(f32) / M, ins['blog'][None, :].astype(f32)])),
        w1a=np.ascontiguousarray(Wg1[:C2]).astype(bf16),
        w1b=np.ascontiguousarray(Wg1[C2:]).astype(bf16),
        bg1=np.ascontiguousarray(ins['bg1'].reshape(H1, 1), f32),
        wg2=np.ascontiguousarray(ins['Wg2'], f32).astype(ml_dtypes.bfloat16),
        bg2_2=np.ascontiguousarray(np.tile(ins['bg2'].astype(f32), 2).reshape(2 * CO, 1)),
        wf1e=np.ascontiguousarray(
            np.vstack([ins['Wf1'].astype(f32), ins['bf1'][None, :].astype(f32)])),
        wf2e=np.ascontiguousarray(
            np.vstack([ins['Wf2'].astype(f32), ins['bf2'].reshape(1, 1).astype(f32)])),
    )
    in_maps = []
    for core in range(N_CORES):
        b, half = core // 2, core % 2
        # odd cores see images in rotated order so the program's local
        # j in {0,1,2} maps to global j in {3,4,5}
        perm = (0, 1, 2, 3, 4, 5) if half == 0 else (3, 4, 5, 0, 1, 2)
        m = dict(common)
        m['patches'] = np.ascontiguousarray(patches[b][:, perm, :]).astype(ml_dtypes.bfloat16)
        m['onehot'] = np.ascontiguousarray(onehots[b][list(perm)])
        in_maps.append(m)
    return in_maps, lab


def _host_post(results, lab):
    P = np.zeros((B, S, S), np.float32)
    cls_terms = np.zeros((B, S), np.float32)
    for core in range(N_CORES):
        b, half = core // 2, core % 2
        perm = (0, 1, 2, 3, 4, 5) if half == 0 else (3, 4, 5, 0, 1, 2)
        sc = results[core]["scores"].reshape(18)
        for jl in range(3):
            for i in range(S):
                P[b, perm[i], perm[jl]] = sc[jl * 6 + i]
        if half == 0:
            cls_terms[b] = results[core]["clsv"].reshape(S)
    cls_loss = np.float32(cls_terms.mean())
    y = (lab[:, :, None] == lab[:, None, :]).astype(np.float32)
    Pt = P.transpose(0, 2, 1)
    sym, anti = np.float32(0.5) * (P + Pt), np.float32(0.5) * (P - Pt)
    sym_n = np.sqrt((sym ** 2).sum(axis=(1, 2)))
    anti_n = np.sqrt((anti ** 2).sum(axis=(1, 2)))
    sym_loss = np.float32(((sym_n - anti_n) / (sym_n + anti_n)).mean())
    euc_loss = np.float32(((P - y) ** 2).mean())
    rn_loss = np.float32(euc_loss - np.float32(0.1) * sym_loss)
    return np.float32(cls_loss), np.float32(rn_loss), np.float32(sym_loss)


def run_spmd(inputs, trace=False, **kwargs):
    nc = _get_nc()
    in_maps, lab = _host_prep(inputs)
    res = run_bass_kernel_spmd(nc, in_maps, list(range(N_CORES)),
                               trace=trace, **kwargs)
    return _host_post(res.results, lab), res


def kernel(**inputs):
    out, _ = run_spmd(inputs)
    return out



# revision 8
# speedup vs baseline: 1.3084x; 1.3084x over previous
"""Trainium2 Bass kernel for nn_Meta_67078799229377 (relation-network meta-learner).

Sharding: 8 cores = 4 batch elements x 2 halves of the relation-j axis.
Each core runs the full backbone for its batch element's 6 images, then the
relation network for its 18 (i, j) pairs, fully fused on-chip (the
[s,s,m,m,128] tensor never exists in HBM).

v2 layout:
  - All constants packed into two HBM tensors (bf16 + f32) -> 4 input DMAs.
  - PE warm-up matmuls at t=0 so the tensor engine is at full p-state when
    conv1 starts.
  - ACT runs Relu only (no activation-table reloads); cls softmax and the
    score-head MLP/sigmoid/losses run on the host from raw logits and the
    128x18 relation sums.
  - relation stage: hdd elementwise generation split DVE/ACT/Pool (tunable),
    g = relu(.)+sum split ACT/Pool via accum_out.
"""
import os
import numpy as np
import ml_dtypes

import concourse.bass as bass
import concourse.mybir as mybir
import concourse.tile as tile
from concourse import bacc
from concourse.bass_utils import run_bass_kernel_spmd

F32 = mybir.dt.float32
BF16 = mybir.dt.bfloat16
AF = mybir.ActivationFunctionType
OP = mybir.AluOpType

B, S, D = 4, 6, 8
M = D * D            # 64 spatial positions
C2 = 66              # 64 channels + 2 coord channels
H1 = 128             # g-MLP hidden
CO = 64              # g-MLP out
NCls = 64
N_CORES = 8

# ---- packed-constant column offsets (bf16 tensor) ----
O_W1 = 0              # [27, 32]
O_W2 = 32             # [32, 9*48]
O_W3 = O_W2 + 432     # [48, 9*64]
O_W1A = O_W3 + 576    # [66, 128]
O_W1B = O_W1A + 128   # [66, 128]
O_WG2 = O_W1B + 128   # [128, 64]
NBF = O_WG2 + 64

# ---- packed-constant column offsets (f32 tensor) ----
OF_BC1 = 0            # [32, 1]
OF_BC2 = 1            # [48, 1]
OF_BC3 = 2            # [64, 1]
OF_BG1 = 3            # [128, 1]
OF_BG2 = 4            # [128, 1] (bg2 duplicated x2)
OF_WLE = 5            # [65, 64]
NF = OF_WLE + 64

# hdd-gen engine assignment per local q (0..31 within a unit); rest on DVE.
# GPSIMD (Pool) cannot touch PSUM, so it only gets SBUF-side hdd work and the
# g-relu/sum instructions are split ACT/DVE.
HDD_ACT_Q = frozenset((7, 15, 23, 31))
HDD_POOL_Q = frozenset()
# g-relu+sum runs on ACT only — DVE accum_out is broken on HW (wrong results,
# 70us/instr) and GPSIMD tensor_scalar runs at ~15ns/row.
# relation blocks: (jl, q_base, n_q); final unit split for a faster drain.
BLOCKS = ((0, 0, 32), (0, 32, 32), (1, 0, 32), (1, 32, 32),
          (2, 0, 32), (2, 32, 16), (2, 48, 16))
NXF = 3 * len(BLOCKS)
N_WARMUP_MM = 14


def _build_nc():
    nc = bacc.Bacc("TRN2", target_bir_lowering=False, debug=False,
                   num_devices=N_CORES)

    x_patches = nc.dram_tensor("patches", [27, S * 1024], BF16, kind="ExternalInput")
    x_cb = nc.dram_tensor("cb", [128, NBF], BF16, kind="ExternalInput")
    x_cf = nc.dram_tensor("cf", [128, NF], F32, kind="ExternalInput")
    x_coords = nc.dram_tensor("coords", [2, S * M], BF16, kind="ExternalInput")

    out_xf = nc.dram_tensor("xf", [128, NXF], F32, kind="ExternalOutput")
    out_logits = nc.dram_tensor("logits", [S, NCls], F32, kind="ExternalOutput")

    with tile.TileContext(nc) as tc:
        with (
            tc.tile_pool(name="const", bufs=1) as cpool,
            tc.tile_pool(name="work", bufs=1) as wpool,
            tc.tile_pool(name="hdd", bufs=2) as hpool,
            tc.tile_pool(name="gscr", bufs=2) as spool,
        ):
            # ---- input DMAs (sliced for parallel transfer) ----
            cb = cpool.tile([128, NBF], BF16)
            nc.sync.dma_start(out=cb[:, 0:O_W1A], in_=x_cb[:, 0:O_W1A])
            patches_sb = cpool.tile([27, S * 1024], BF16)
            for g in range(3):
                nc.sync.dma_start(out=patches_sb[:, g * 2048:(g + 1) * 2048],
                                  in_=x_patches[:, g * 2048:(g + 1) * 2048])
            nc.sync.dma_start(out=cb[:, O_W1A:NBF], in_=x_cb[:, O_W1A:NBF])
            cf = cpool.tile([128, NF], F32)
            nc.sync.dma_start(out=cf[:], in_=x_cf[:])
            featc = wpool.tile([C2, S * M], BF16)
            nc.sync.dma_start(out=featc[64:66, :], in_=x_coords[:])

            w1 = cb[0:27, O_W1:O_W1 + 32]
            w2 = cb[0:32, O_W2:O_W2 + 432]
            w3 = cb[0:48, O_W3:O_W3 + 576]
            w1a = cb[0:C2, O_W1A:O_W1A + 128]
            w1b = cb[0:C2, O_W1B:O_W1B + 128]
            wg2 = cb[0:H1, O_WG2:O_WG2 + 64]
            bc1 = cf[0:32, OF_BC1:OF_BC1 + 1]
            bc2 = cf[0:48, OF_BC2:OF_BC2 + 1]
            bc3 = cf[0:64, OF_BC3:OF_BC3 + 1]
            bg1 = cf[0:H1, OF_BG1:OF_BG1 + 1]
            bg2 = cf[0:128, OF_BG2:OF_BG2 + 1]
            wle = cf[0:65, OF_WLE:OF_WLE + 64]

            c1sb = wpool.tile([32, S, 33, 33], BF16)
            c2sb = wpool.tile([48, S, 17, 17], BF16)
            xf_cols = wpool.tile([2 * CO, NXF], F32)
            u_f32 = wpool.tile([H1, 3 * M], F32)
            v_bf = wpool.tile([H1, S * M], BF16)
            fme = wpool.tile([65, S], F32)
            logits_sb = wpool.tile([S, NCls], F32)
            wu = wpool.tile([128, 512], BF16)

            # padding strips + warm-up source (all on Pool; it is idle anyway)
            nc.gpsimd.memset(wu[:], 0.0)
            for img in range(S):
                nc.gpsimd.memset(c1sb[:, img, 32, :], 0.0)
                nc.gpsimd.memset(c1sb[:, img, 0:32, 32], 0.0)
                nc.gpsimd.memset(c2sb[:, img, 16, :], 0.0)
                nc.gpsimd.memset(c2sb[:, img, 0:16, 16], 0.0)
            nc.gpsimd.memset(fme[64:65, :], 1.0)

            # ================= conv + cls + uv scope =================
            with (
                tc.tile_pool(name="pc1", bufs=2, space="PSUM") as p1_pool,
                tc.tile_pool(name="pc2", bufs=2, space="PSUM") as p2_pool,
                tc.tile_pool(name="psm", bufs=2, space="PSUM") as ps_pool,
            ):
                # PE warm-up: ramp the p-state while DMAs land.
                wu_ps = ps_pool.tile([128, 512], F32, tag="sm")
                for _ in range(N_WARMUP_MM):
                    nc.tensor.matmul(wu_ps[:], wu[:, 0:128], wu[:],
                                     start=True, stop=True)

                # ---- conv1: [27]->[32], 64x64 -> 32x32 (im2col'd) ----
                for img in range(S):
                    ps1 = p1_pool.tile([32, 2, 16, 32], F32, tag="c1")
                    for h in range(2):
                        nc.tensor.matmul(
                            ps1[:, h, :, :].rearrange("p a b -> p (a b)"),
                            w1,
                            patches_sb[:, img * 1024 + h * 512:
                                       img * 1024 + (h + 1) * 512],
                            start=True, stop=True)
                    out_ap = c1sb[:, img, 0:32, 0:32]
                    if img % 2 == 0:
                        nc.vector.tensor_scalar(out_ap, ps1[:], bc1, 0.0,
                                                op0=OP.add, op1=OP.max)
                    else:
                        nc.scalar.activation(out_ap, ps1[:], AF.Relu, bias=bc1)

                # ---- conv2: [32]->[48], 32x32 -> 16x16 ----
                for ip in range(3):      # image pairs
                    ps2 = p2_pool.tile([48, 2, 16, 16], F32, tag="c2")
                    for k, (dy, dx) in enumerate(
                            (dy, dx) for dy in range(3) for dx in range(3)):
                        nc.tensor.matmul(
                            ps2[:],
                            w2[:, k * 48:(k + 1) * 48],
                            c1sb[:, 2 * ip:2 * ip + 2, dy:dy + 31:2, dx:dx + 31:2],
                            start=(k == 0), stop=(k == 8))
                    out_ap = c2sb[:, 2 * ip:2 * ip + 2, 0:16, 0:16]
                    if ip != 1:
                        nc.vector.tensor_scalar(out_ap, ps2[:], bc2, 0.0,
                                                op0=OP.add, op1=OP.max)
                    else:
                        nc.scalar.activation(out_ap, ps2[:], AF.Relu, bias=bc2)

                # ---- conv3: [48]->[64], 16x16 -> 8x8 ----
                ps3 = ps_pool.tile([64, S, D, D], F32, tag="sm")
                for k, (dy, dx) in enumerate(
                        (dy, dx) for dy in range(3) for dx in range(3)):
                    nc.tensor.matmul(
                        ps3[:],
                        w3[:, k * 64:(k + 1) * 64],
                        c2sb[:, :, dy:dy + 15:2, dx:dx + 15:2],
                        start=(k == 0), stop=(k == 8))
                nc.scalar.activation(featc[0:64, :].rearrange("p (i m) -> p i m", m=M),
                                     ps3[:].rearrange("p i a b -> p i (a b)"),
                                     AF.Relu, bias=bc3)

                # ---- cls head: raw logits only ----
                nc.vector.tensor_reduce(
                    fme[0:64, :], featc[0:64, :].rearrange("p (i m) -> p i m", m=M),
                    axis=mybir.AxisListType.X, op=OP.add)
                psl = ps_pool.tile([S, NCls], F32, tag="sm")
                nc.tensor.matmul(psl[:], fme[:], wle, start=True, stop=True)
                nc.vector.tensor_copy(logits_sb[:], psl[:])
                nc.sync.dma_start(out=out_logits[:], in_=logits_sb[:])

                # ---- u / v ----
                psu = ps_pool.tile([H1, S * M], F32, tag="sm")
                psv = ps_pool.tile([H1, S * M], F32, tag="sm")
                nc.tensor.matmul(psu[:], w1a, featc[:], start=True, stop=True)
                nc.tensor.matmul(psv[:], w1b, featc[:], start=True, stop=True)
                nc.vector.tensor_copy(u_f32[:], psu[:, 0:3 * M])
                nc.vector.tensor_scalar(v_bf[:], psv[:], bg1, None, op0=OP.add)

            # ================= relation scope =================
            with tc.tile_pool(name="pbig", bufs=2, space="PSUM") as pb_pool:
                for bi, (jl, qb, nq) in enumerate(BLOCKS):
                    hdd = hpool.tile([H1, nq, S * M], BF16, tag="hdd")
                    for ql in range(nq):
                        q = qb + ql
                        ucol = u_f32[:, jl * M + q: jl * M + q + 1]
                        if (ql % 32) in HDD_ACT_Q:
                            nc.scalar.activation(hdd[:, ql, :], v_bf[:],
                                                 AF.Relu, bias=ucol)
                        else:
                            nc.vector.tensor_scalar(hdd[:, ql, :], v_bf[:],
                                                    ucol, 0.0,
                                                    op0=OP.add, op1=OP.max)
                    for duo in range(3):
                        iA, iB = 2 * duo, 2 * duo + 1
                        ps = pb_pool.tile([2 * CO, nq * 64], F32, tag="gps")
                        for qg in range(nq // 8):
                            nc.tensor.matmul(
                                ps[0:CO, qg * 512:(qg + 1) * 512],
                                wg2,
                                hdd[:, qg * 8:(qg + 1) * 8, iA * M:(iA + 1) * M],
                                start=True, stop=True)
                            nc.tensor.matmul(
                                ps[CO:2 * CO, qg * 512:(qg + 1) * 512],
                                wg2,
                                hdd[:, qg * 8:(qg + 1) * 8, iB * M:(iB + 1) * M],
                                start=True, stop=True,
                                tile_position=(0, 64))
                        col = 3 * bi + duo
                        gscr = spool.tile([2 * CO, nq * 64], BF16, tag="gscr")
                        nc.scalar.activation(
                            gscr[:], ps[:], AF.Relu, bias=bg2,
                            accum_out=xf_cols[:, col:col + 1])

            nc.sync.dma_start(out=out_xf[:], in_=xf_cols[:])
    nc.compile()
    return nc


_NC_CACHE = None


def _get_nc():
    global _NC_CACHE
    if _NC_CACHE is None:
        _NC_CACHE = _build_nc()
    return _NC_CACHE


def _host_prep(inputs):
    ins = {k: np.asarray(v) for k, v in inputs.items()}
    x = np.concatenate([ins['support_x'], ins['query_x']], axis=1)
    lab = np.concatenate([ins['support_y'], ins['query_y']], axis=1)

    xpad = np.pad(x.astype(np.float32), ((0, 0), (0, 0), (0, 0), (0, 1), (0, 1)))
    win = np.lib.stride_tricks.sliding_window_view(xpad, (3, 3), axis=(3, 4))
    win = win[:, :, :, ::2, ::2]
    patches = win.transpose(0, 2, 5, 6, 1, 3, 4).reshape(B, 27, S, 1024)
    patches = np.ascontiguousarray(patches, np.float32)

    f32 = np.float32
    bf16 = ml_dtypes.bfloat16

    cbf = np.zeros((128, NBF), f32)
    cbf[0:27, O_W1:O_W1 + 32] = ins['k1'].reshape(32, 27).T
    cbf[0:32, O_W2:O_W2 + 432] = ins['k2'].transpose(1, 2, 3, 0).reshape(32, 432)
    cbf[0:48, O_W3:O_W3 + 576] = ins['k3'].transpose(1, 2, 3, 0).reshape(48, 576)
    Wg1 = ins['Wg1'].astype(f32)
    cbf[0:C2, O_W1A:O_W1A + 128] = Wg1[:C2]
    cbf[0:C2, O_W1B:O_W1B + 128] = Wg1[C2:]
    cbf[0:H1, O_WG2:O_WG2 + 64] = ins['Wg2']
    cbf = cbf.astype(bf16)

    cff = np.zeros((128, NF), f32)
    cff[0:32, OF_BC1] = ins['bc1']
    cff[0:48, OF_BC2] = ins['bc2']
    cff[0:64, OF_BC3] = ins['bc3']
    cff[0:H1, OF_BG1] = ins['bg1']
    cff[0:128, OF_BG2] = np.tile(ins['bg2'].astype(f32), 2)
    cff[0:65, OF_WLE:OF_WLE + 64] = np.vstack(
        [ins['Wlog'].astype(f32) / M, ins['blog'][None, :].astype(f32)])

    ii = np.arange(D, dtype=f32) / D
    coord = np.stack([np.broadcast_to(ii[:, None], (D, D)),
                      np.broadcast_to(ii[None, :], (D, D))]).reshape(2, M)
    coords = np.ascontiguousarray(np.tile(coord, (1, S)), f32).astype(bf16)

    common = dict(cb=cbf, cf=np.ascontiguousarray(cff), coords=coords)
    in_maps = []
    for core in range(N_CORES):
        b, half = core // 2, core % 2
        # odd cores see images in rotated order so the program's local
        # j in {0,1,2} maps to global j in {3,4,5}
        perm = (0, 1, 2, 3, 4, 5) if half == 0 else (3, 4, 5, 0, 1, 2)
        m = dict(common)
        m['patches'] = np.ascontiguousarray(
            patches[b][:, perm, :]).reshape(27, S * 1024).astype(bf16)
        in_maps.append(m)
    return in_maps, lab, ins


def _host_post(results, lab, ins):
    f32 = np.float32
    # ---- cls loss from raw logits (even cores have identity perm) ----
    cls_terms = np.zeros((B, S), f32)
    for b in range(B):
        logits = results[2 * b]["logits"].astype(f32)          # [6, 64]
        mx = logits.max(axis=1, keepdims=True)
        lse = np.log(np.exp(logits - mx).sum(axis=1, keepdims=True)) + mx
        logp = logits - lse
        cls_terms[b] = lse[:, 0] - logits[np.arange(S), lab[b]]
    cls_loss = np.float32(cls_terms.mean())

    # ---- score head from relation sums ----
    xf = np.zeros((B, S, S, 2 * CO), f32)   # [b, i_loc?, ...]
    P = np.zeros((B, S, S), f32)
    Wf1, bf1 = ins['Wf1'].astype(f32), ins['bf1'].astype(f32)
    Wf2, bf2 = ins['Wf2'].astype(f32), ins['bf2'].astype(f32)
    for core in range(N_CORES):
        b, half = core // 2, core % 2
        perm = (0, 1, 2, 3, 4, 5) if half == 0 else (3, 4, 5, 0, 1, 2)
        dev = results[core]["xf"].astype(f32)                  # [128, NXF]
        for jl in range(3):
            for duo in range(3):
                colsum = np.zeros(128, f32)
                for bi, (bjl, _, _) in enumerate(BLOCKS):
                    if bjl == jl:
                        colsum += dev[:, 3 * bi + duo]
                for ih in range(2):
                    i_loc = 2 * duo + ih
                    x_f = colsum[ih * CO:(ih + 1) * CO]
                    h = np.maximum(x_f @ Wf1 + bf1, 0.0)
                    s2 = h @ Wf2 + bf2
                    P[b, perm[i_loc], perm[jl]] = 1.0 / (1.0 + np.exp(-s2[0]))

    y = (lab[:, :, None] == lab[:, None, :]).astype(f32)
    Pt = P.transpose(0, 2, 1)
    sym, anti = f32(0.5) * (P + Pt), f32(0.5) * (P - Pt)
    sym_n = np.sqrt((sym ** 2).sum(axis=(1, 2)))
    anti_n = np.sqrt((anti ** 2).sum(axis=(1, 2)))
    sym_loss = np.float32(((sym_n - anti_n) / (sym_n + anti_n)).mean())
    euc_loss = np.float32(((P - y) ** 2).mean())
    rn_loss = np.float32(euc_loss - np.float32(0.1) * sym_loss)
    return np.float32(cls_loss), np.float32(rn_loss), np.float32(sym_loss)


def run_spmd(inputs, trace=False, **kwargs):
    nc = _get_nc()
    in_maps, lab, ins = _host_prep(inputs)
    res = run_bass_kernel_spmd(nc, in_maps, list(range(N_CORES)),
                               trace=trace, **kwargs)
    return _host_post(res.results, lab, ins), res


def kernel(**inputs):
    out, _ = run_spmd(inputs)
    return out
